# revision 3
# baseline (speedup 1.0000x reference)
"""Trainium2 Bass kernel for the Axis-Portrait-Attention block.

Reference computation (per batch b of 4):
  Q/K/V = relu(1x1conv(x))           # [256, 128, 128]
  Qm = raw-reshape -> [128, 32768]   # rows are channel PAIRS, cols j=(c%2,y,x)
  logits = Qm @ Km^T                 # [128, 128], contraction over j
  attn = softmax(logits, axis=0 per column)
  out1[j, i2] = sum_i1 Vm[i1, j] attn[i1, i2]
  out_w = Wu @ (gamma * out1 raw-reshaped [256,128,128]) + bu

Sharding over 8 cores: core k -> (batch b=k//2, half h=k%2).
  - Q/K convs + partial logits over yv-half rows; logits pair-AllReduced.
  - V conv / out1 / up-conv over xv-half cols (xv maps to the output's H
    index under the raw reshape, so each core owns a clean out_w slab and
    the up-conv channel contraction stays fully local).

Precision: Q/K convs + logits matmul in fp32 (softmax columns can have
top-2 logit gaps < 1, fp22 conv noise visibly perturbs attn there); all
other matmuls in float32r (fp22 multiplies, full PE rate at N>=256).
"""

import sys

sys.path.insert(0, "/opt/trn_rl_repo")

import numpy as np

B, C, H, W = 4, 256, 128, 128
Cr = C
N_CORES = 8
HALF = 64  # spatial half size

_BUILT = None


def _build():
    import concourse.bass as bass  # noqa: F401
    import concourse.tile as tile
    from concourse import bacc, mybir

    f32 = mybir.dt.float32
    f32r = mybir.dt.float32r
    AF = mybir.ActivationFunctionType
    X_AXIS = mybir.AxisListType.X

    nc = bacc.Bacc(
        "TRN2", target_bir_lowering=False, debug=False, num_devices=N_CORES
    )

    # ---- dram parameters (inputs) ----
    dp = nc.declare_dram_parameter
    x_yv = dp("x_yv", [C, 64 * 128], f32, isOutput=False)  # [c, yv-half*xv]
    x_xv = dp("x_xv", [C, 128 * 64], f32, isOutput=False)  # [c, yv*xvl-half]
    wqt = dp("wqt", [C, C], f32, isOutput=False)  # Wq[perm].T (in, out-perm)
    wkt = dp("wkt", [C, C], f32, isOutput=False)
    wvt = dp("wvt", [C, C], f32, isOutput=False)
    wut = dp("wut", [C, C], f32, isOutput=False)  # Wu.T (c_r, o2)
    bqb = dp("bqb", [128, C], f32, isOutput=False)  # bq[perm] bcast over parts
    bkb = dp("bkb", [128, C], f32, isOutput=False)
    bvb = dp("bvb", [128, C], f32, isOutput=False)
    bqp = dp("bqp", [C, 1], f32, isOutput=False)  # bq[perm] per-partition
    bvp = dp("bvp", [C, 1], f32, isOutput=False)
    bup = dp("bup", [C, 1], f32, isOutput=False)  # bu per-partition
    gam = dp("gam", [128, 1], f32, isOutput=False)  # gamma replicated
    ident = dp("ident", [128, 128], f32, isOutput=False)

    # ---- dram parameters (outputs) ----
    q_out = dp("q_out", [128, 2, 64 * 128], f32, isOutput=True)  # (i1,p,yv*xv)
    k_out = dp("k_out", [2, 64 * 128, 128], f32, isOutput=True)  # (p,yv*xv,i2)
    v_out = dp("v_out", [2, 128 * 64, 128], f32, isOutput=True)  # (p,yv*xvl,i)
    ow_out = dp("ow_out", [C, 64 * 128], f32, isOutput=True)  # (o2, yo*xo)
    attn_out = dp("attn_out", [128, 128], f32, isOutput=True)

    with tile.TileContext(nc) as tc:
        with (
            tc.tile_pool(name="consts", bufs=1) as cp,
            tc.tile_pool(name="dram", bufs=1, space="DRAM") as dram,
            tc.tile_pool(name="smx", bufs=1) as smx,
        ):
            # ---- load constants ----
            def load_pair(param, name, dt):
                ts = []
                for ct in range(2):
                    t = cp.tile([128, C], dt, name=f"{name}{ct}")
                    src_ap = param[ct * 128 : (ct + 1) * 128, :]
                    if dt is f32r:
                        src_ap = src_ap.bitcast(f32r)
                    nc.sync.dma_start(out=t[:], in_=src_ap)
                    ts.append(t)
                return ts

            wq_sb = load_pair(wqt, "wq", f32)
            wk_sb = load_pair(wkt, "wk", f32)
            wq_r = load_pair(wqt, "wqr", f32r)
            wv_sb = load_pair(wvt, "wv", f32r)
            wu_sb = load_pair(wut, "wu", f32r)

            def load_col(param, name):
                ts = []
                for ot in range(2):
                    t = cp.tile([128, 1], f32, name=f"{name}{ot}")
                    nc.sync.dma_start(
                        out=t[:], in_=param[ot * 128 : (ot + 1) * 128, :]
                    )
                    ts.append(t)
                return ts

            bqp_sb = load_col(bqp, "bqp")
            bvp_sb = load_col(bvp, "bvp")
            bup_sb = load_col(bup, "bup")

            bqb_sb = cp.tile([128, C], f32)
            nc.sync.dma_start(out=bqb_sb[:], in_=bqb[:, :])
            bkb_sb = cp.tile([128, C], f32)
            nc.sync.dma_start(out=bkb_sb[:], in_=bkb[:, :])
            bvb_sb = cp.tile([128, C], f32)
            nc.sync.dma_start(out=bvb_sb[:], in_=bvb[:, :])
            gam_sb = cp.tile([128, 1], f32)
            nc.sync.dma_start(out=gam_sb[:], in_=gam[:, :])
            ident_sb = cp.tile([128, 128], f32)
            nc.sync.dma_start(out=ident_sb[:], in_=ident[:, :])

            cc_in = dram.tile([128, 128], f32)
            cc_out = dram.tile([128, 128], f32)

            # ================= phase 1: Q/K convs + partial logits ========
            with (
                tc.tile_pool(name="xp", bufs=3) as xp,
                tc.tile_pool(name="qkps", bufs=2, space="PSUM") as qkps,
                tc.tile_pool(name="lgps", bufs=1, space="PSUM") as lgps,
                tc.tile_pool(name="qks", bufs=4) as qks,
                tc.tile_pool(name="wsps", bufs=2, space="PSUM") as wsps,
                tc.tile_pool(name="wssb", bufs=3) as wssb,
            ):
                lg_ps = lgps.tile([128, 128], f32)
                nmm = 0
                for sc in range(16):
                    xts = []
                    xtr = []
                    for ct in range(2):
                        xt = xp.tile([128, 512], f32, tag="x", name=f"xt{ct}")
                        nc.sync.dma_start(
                            out=xt[:],
                            in_=x_yv[
                                ct * 128 : (ct + 1) * 128,
                                sc * 512 : (sc + 1) * 512,
                            ],
                        )
                        xts.append(xt)
                        xr = xp.tile([128, 512], f32r, tag="xr", name=f"xr{ct}")
                        nc.gpsimd.tensor_copy(xr[:], xt[:])
                        xtr.append(xr)
                    # weight-stationary Q conv (f32r): psum [i-chan, 512 s]
                    for ot in range(2):
                        ps = wsps.tile([128, 512], f32, tag="qws_ps")
                        for ct in range(2):
                            nc.tensor.matmul(
                                ps[:],
                                lhsT=wq_r[ct][:, ot * 128 : (ot + 1) * 128],
                                rhs=xtr[ct][:],
                                start=(ct == 0),
                                stop=(ct == 1),
                            )
                        qt = wssb.tile([128, 512], f32, tag="qws_sb")
                        nc.scalar.activation(
                            qt[:], ps[:], AF.Relu, bias=bqp_sb[ot][:], scale=1.0
                        )
                        nc.sync.dma_start(
                            out=q_out[:, ot, sc * 512 : (sc + 1) * 512],
                            in_=qt[:],
                        )
                    # x-stationary Q/K convs (fp32) + partial logits (fp32)
                    for st in range(4):
                        qps = qkps.tile([128, 256], f32, tag="qxs")
                        kps = qkps.tile([128, 256], f32, tag="kxs")
                        for ct in range(2):
                            nc.tensor.matmul(
                                qps[:],
                                lhsT=xts[ct][:, st * 128 : (st + 1) * 128],
                                rhs=wq_sb[ct][:],
                                start=(ct == 0),
                                stop=(ct == 1),
                            )
                        for ct in range(2):
                            nc.tensor.matmul(
                                kps[:],
                                lhsT=xts[ct][:, st * 128 : (st + 1) * 128],
                                rhs=wk_sb[ct][:],
                                start=(ct == 0),
                                stop=(ct == 1),
                            )
                        qxt = qks.tile([128, 256], f32, tag="qxt")
                        kxt = qks.tile([128, 256], f32, tag="kxt")
                        nc.vector.tensor_add(qxt[:], qps[:], bqb_sb[:])
                        nc.gpsimd.tensor_scalar_max(qxt[:], qxt[:], 0.0)
                        nc.vector.tensor_add(kxt[:], kps[:], bkb_sb[:])
                        nc.gpsimd.tensor_scalar_max(kxt[:], kxt[:], 0.0)
                        sg = sc * 4 + st
                        for p in range(2):
                            nc.sync.dma_start(
                                out=k_out[p, sg * 128 : (sg + 1) * 128, :],
                                in_=kxt[:, p * 128 : (p + 1) * 128],
                            )
                            nc.tensor.matmul(
                                lg_ps[:],
                                lhsT=kxt[:, p * 128 : (p + 1) * 128],
                                rhs=qxt[:, p * 128 : (p + 1) * 128],
                                start=(nmm == 0),
                                stop=(nmm == 127),
                            )
                            nmm += 1
                # partial logitsT -> AllReduce within the batch pair
                lg_sb = smx.tile([128, 128], f32)
                nc.vector.tensor_copy(lg_sb[:], lg_ps[:])
                nc.sync.dma_start(out=cc_in[:], in_=lg_sb[:])
                nc.gpsimd.collective_compute(
                    "AllReduce",
                    mybir.AluOpType.add,
                    replica_groups=[[0, 1], [2, 3], [4, 5], [6, 7]],
                    ins=[cc_in.opt()],
                    outs=[cc_out.opt()],
                )

            # ================= softmax over i1 (free axis of lgT) =========
            with tc.tile_pool(name="smxps", bufs=1, space="PSUM") as smxps:
                lgT = smx.tile([128, 128], f32)
                nc.sync.dma_start(out=lgT[:], in_=cc_out[:])
                mx = smx.tile([128, 1], f32)
                nc.vector.reduce_max(mx[:], lgT[:], axis=X_AXIS, negate=True)
                et = smx.tile([128, 128], f32)
                nc.scalar.activation(et[:], lgT[:], AF.Exp, bias=mx[:], scale=1.0)
                sm = smx.tile([128, 1], f32)
                nc.vector.reduce_sum(sm[:], et[:], axis=X_AXIS)
                rc = smx.tile([128, 1], f32)
                nc.vector.reciprocal(rc[:], sm[:])
                attnT = smx.tile([128, 128], f32)
                nc.vector.tensor_scalar_mul(attnT[:], et[:], rc[:])
                aps = smxps.tile([128, 128], f32)
                nc.tensor.transpose(aps[:], attnT[:], ident_sb[:])
                attn_sb = smx.tile([128, 128], f32)
                nc.vector.tensor_copy(attn_sb[:], aps[:])
                nc.sync.dma_start(out=attn_out[:, :], in_=attn_sb[:])
                attn_g = smx.tile([128, 128], f32)
                nc.vector.tensor_scalar_mul(attn_g[:], attn_sb[:], gam_sb[:])

            # ================= phase 2: V conv, out1, up-conv =============
            with (
                tc.tile_pool(name="xvp", bufs=1) as xvp,
                tc.tile_pool(name="vxps", bufs=2, space="PSUM") as vxps,
                tc.tile_pool(name="vxsb", bufs=4) as vxsb,
                tc.tile_pool(name="vwps", bufs=2, space="PSUM") as vwps,
                tc.tile_pool(name="vwsb", bufs=3) as vwsb,
                tc.tile_pool(name="o1ps", bufs=2, space="PSUM") as o1ps,
                tc.tile_pool(name="omp", bufs=3) as omp,
                tc.tile_pool(name="owps", bufs=2, space="PSUM") as owps,
                tc.tile_pool(name="owsb", bufs=3) as owsb,
            ):
                xv_sb = []
                for ct in range(2):
                    t = xvp.tile([128, 8192], f32r, name=f"xv{ct}")
                    nc.sync.dma_start(
                        out=t[:],
                        in_=x_xv[ct * 128 : (ct + 1) * 128, :].bitcast(f32r),
                    )
                    xv_sb.append(t)
                # x-stationary V conv (f32r), (yv,xvl)-ordered spatial
                for sc in range(16):
                    for st in range(4):
                        vps = vxps.tile([128, 256], f32, tag="vxs")
                        for ct in range(2):
                            nc.tensor.matmul(
                                vps[:],
                                lhsT=xv_sb[ct][
                                    :, (sc * 4 + st) * 128 : (sc * 4 + st + 1) * 128
                                ],
                                rhs=wv_sb[ct][:],
                                start=(ct == 0),
                                stop=(ct == 1),
                            )
                        vxt = vxsb.tile([128, 256], f32, tag="vxt")
                        nc.vector.tensor_add(vxt[:], vps[:], bvb_sb[:])
                        nc.gpsimd.tensor_scalar_max(vxt[:], vxt[:], 0.0)
                        sg = sc * 4 + st
                        for p in range(2):
                            nc.sync.dma_start(
                                out=v_out[p, sg * 128 : (sg + 1) * 128, :],
                                in_=vxt[:, p * 128 : (p + 1) * 128],
                            )
                # weight-stationary V conv + out1 + up-conv, per xvl-group
                for wsc in range(16):
                    vws = []
                    for ot in range(2):
                        ps = vwps.tile([128, 512], f32, tag="vws_ps")
                        for ct in range(2):
                            rhs = xv_sb[ct][:, :].rearrange(
                                "c (y x) -> c x y", x=64
                            )[:, wsc * 4 : (wsc + 1) * 4, :]
                            nc.tensor.matmul(
                                ps[:],
                                lhsT=wv_sb[ct][:, ot * 128 : (ot + 1) * 128],
                                rhs=rhs,
                                start=(ct == 0),
                                stop=(ct == 1),
                            )
                        vt = vwsb.tile([128, 512], f32, tag="vws_sb")
                        nc.scalar.activation(
                            vt[:], ps[:], AF.Relu, bias=bvp_sb[ot][:], scale=1.0
                        )
                        vws.append(vt)
                    omch = []
                    for pt in range(2):
                        om = omp.tile([128, 512], f32r, tag=f"om{pt}")
                        omch.append(om)
                    for xvl in range(4):
                        for pt in range(2):
                            ops_ = o1ps.tile([128, 128], f32, tag="o1")
                            nc.tensor.matmul(
                                ops_[:],
                                lhsT=vws[pt][:, xvl * 128 : (xvl + 1) * 128],
                                rhs=attn_g[:],
                                start=True,
                                stop=True,
                            )
                            nc.vector.tensor_copy(
                                omch[pt][:, xvl * 128 : (xvl + 1) * 128], ops_[:]
                            )
                    for ot2 in range(2):
                        ps = owps.tile([128, 512], f32, tag="ow_ps")
                        for crt in range(2):
                            nc.tensor.matmul(
                                ps[:],
                                lhsT=wu_sb[crt][:, ot2 * 128 : (ot2 + 1) * 128],
                                rhs=omch[crt][:],
                                start=(crt == 0),
                                stop=(crt == 1),
                            )
                        owt = owsb.tile([128, 512], f32, tag="ow_sb")
                        nc.vector.tensor_scalar_add(owt[:], ps[:], bup_sb[ot2][:])
                        nc.sync.dma_start(
                            out=ow_out[
                                ot2 * 128 : (ot2 + 1) * 128,
                                wsc * 512 : (wsc + 1) * 512,
                            ],
                            in_=owt[:],
                        )

    nc.compile()
    return nc


def _get_built():
    global _BUILT
    if _BUILT is None:
        _BUILT = _build()
    return _BUILT


def _make_in_maps(x, Wq, bq, Wk, bk, Wv, bv, Wu, bu, gamma):
    perm = np.concatenate([np.arange(0, C, 2), np.arange(1, C, 2)])
    f = np.float32
    wqt = np.ascontiguousarray(Wq[perm].T, dtype=f)
    wkt = np.ascontiguousarray(Wk[perm].T, dtype=f)
    wvt = np.ascontiguousarray(Wv[perm].T, dtype=f)
    wut = np.ascontiguousarray(Wu.T, dtype=f)
    bqb = np.tile(bq[perm][None, :], (128, 1)).astype(f)
    bkb = np.tile(bk[perm][None, :], (128, 1)).astype(f)
    bvb = np.tile(bv[perm][None, :], (128, 1)).astype(f)
    bqp = bq[perm].reshape(C, 1).astype(f)
    bvp = bv[perm].reshape(C, 1).astype(f)
    bup = bu.reshape(C, 1).astype(f)
    gamv = np.full((128, 1), float(np.asarray(gamma).reshape(-1)[0]), dtype=f)
    identm = np.eye(128, dtype=f)
    common = dict(
        wqt=wqt, wkt=wkt, wvt=wvt, wut=wut,
        bqb=bqb, bkb=bkb, bvb=bvb,
        bqp=bqp, bvp=bvp, bup=bup,
        gam=gamv, ident=identm,
    )
    in_maps = []
    for k in range(N_CORES):
        b, h = k // 2, k % 2
        y0 = h * HALF
        x0 = h * HALF
        x_yv = np.ascontiguousarray(
            x[b, :, y0 : y0 + HALF, :], dtype=f
        ).reshape(C, HALF * W)
        x_xv = np.ascontiguousarray(
            x[b, :, :, x0 : x0 + HALF], dtype=f
        ).reshape(C, H * HALF)
        in_maps.append(dict(common, x_yv=x_yv, x_xv=x_xv))
    return in_maps


def run_cores(in_maps, trace=False, **kw):
    from concourse.bass_utils import run_bass_kernel_spmd

    nc = _get_built()
    return run_bass_kernel_spmd(
        nc, in_maps, list(range(N_CORES)), trace=trace, **kw
    )


def assemble(results, gamma):
    f = np.float32
    ow = np.empty((B, C, H, W), dtype=f)
    xQw = np.empty((B, 128, 2, 128, 128), dtype=f)
    xKw = np.empty((B, 2, 128, 128, 128), dtype=f)
    xVw = np.empty((B, 2, 128, 128, 128), dtype=f)
    attn = np.empty((B, 128, 128), dtype=f)
    for k in range(N_CORES):
        b, h = k // 2, k % 2
        y0 = h * HALF
        x0 = h * HALF
        r = results[k]
        xQw[b][:, :, y0 : y0 + HALF, :] = r["q_out"].reshape(128, 2, HALF, 128)
        xKw[b][:, y0 : y0 + HALF, :, :] = r["k_out"].reshape(2, HALF, 128, 128)
        xVw[b][:, :, x0 : x0 + HALF, :] = r["v_out"].reshape(2, 128, HALF, 128)
        ow[b][:, x0 : x0 + HALF, :] = r["ow_out"].reshape(C, HALF, 128)
        if h == 0:
            attn[b] = r["attn_out"]
    g = np.asarray(gamma, dtype=f).reshape(1)
    return (
        ow,
        np.ascontiguousarray(xQw.reshape(B, 128, 32768)),
        np.ascontiguousarray(xKw.reshape(B, 32768, 128)),
        np.ascontiguousarray(xVw.reshape(B, 32768, 128)),
        g,
        attn,
    )


def kernel(x, Wq, bq, Wk, bk, Wv, bv, Wu, bu, gamma):
    x = np.asarray(x, dtype=np.float32)
    in_maps = _make_in_maps(x, Wq, bq, Wk, bk, Wv, bv, Wu, bu, gamma)
    res = run_cores(in_maps)
    return assemble(res.results, gamma)
